# revision 35
# baseline (speedup 1.0000x reference)
"""Causal scaled-dot-product attention for Trainium2 (Bass/Tile), 8-core SPMD.

Problem: B=2, H=16, S=2048, D=128 fp32, causal mask, softmax(QK^T/sqrt(D)) @ V.
Sharding: batch*heads (32) split across 8 cores, 4 heads per core; attention is
independent per (b,h) so there is no communication.

Design (bf16 on the PE; ~1.6x over the f32r predecessor, ~109-113us):
  - Host casts Q,K,V to bf16 and appends a ones-column to V (V1 = [V | 1]).
  - Q^T,K^T loaded straight into SBUF via 2-byte DMA xbar transpose
    (dma_start_transpose) -> zero PE transposes.  Whole-tensor loads with
    2-head prefetch only: just-in-time split pieces raced on HW.
  - Per 512-wide query chunk, key tiles are processed in descending-j groups
    of 4 (psA, 4 PSUM banks) strictly alternating with 2 (psB, 2 banks):
      S^T[j] = K_j @ Q_c^T          (bf16 matmul, 1 col/cycle; fp32r is 2)
      one merged exp per group      (ACT, PSUM->SBUF bf16; trimmed cols of
                                     later slots exp junk that is never read)
      diagonal 128-blocks masked in place on DVE with a bf16 upper-tri const
  - PV uses pexp as the *stationary* operand and V1 as the moving operand:
      OUT[qtile, 0:129] += pexp_j,t^T @ [V_j | 1]
    so the output lands directly in [q, d] layout (no output transpose) and
    column 128 accumulates the softmax denominator for free.
  - Tail per chunk: reciprocal of den, per-partition scale, DMA out (gpsimd
    swdge queue, so stores never alias the sync queue's load semaphores).
Softmax max-subtraction is skipped: logits are bounded (~±6) so exp is safe,
and softmax is shift-invariant.

Steady state is ACT(exp)-bound (~77us busy) with the PE close behind (~72us,
PV is LDWEIGHTS-rate-limited); PV is deferred by exactly two groups (three
broke: intermittent races), heads end on a small chunk to avoid PV-backlog
bubbles at head boundaries, and the whole kernel drains on chunk c0.
"""
from collections import deque

import numpy as np
import ml_dtypes

import concourse.bacc as bacc
import concourse.tile as tile
import concourse.mybir as mybir
from concourse.bass_utils import run_bass_kernel_spmd
from concourse.masks import make_upper_triangular

F32 = mybir.dt.float32
BF16 = mybir.dt.bfloat16
EXP = mybir.ActivationFunctionType.Exp

B, H, S, D = 2, 16, 2048, 128
TEMPERATURE = 11.313708498984761  # sqrt(128)
N_CORES = 8
HEADS_PER_CORE = (B * H) // N_CORES  # 4
P = 128                    # partitions / tile edge
CHUNK = 512                # query chunk
N_KT = S // P              # 16 key tiles per head
N_CH = S // CHUNK          # 4 query chunks per head
DV = 132                   # V free size: 128 d + 1 ones + 3 pad
# psO slot layout: per-qtile [q,129] accumulation regions, each within a
# single 2KB PSUM bank (bank0: t0..t2, bank1: t3).  start_tensor_calc marks
# the WHOLE bank pending-zero, so start=True is only emitted on the first
# write to each bank per chunk (t3's and t2's diag matmuls); first writes to
# the other regions rely on the bank-wide pending-zero to land as overwrites.
PSO_OFF = (0, 132, 264, 512)


def build_attention_nc():
    nc = bacc.Bacc("TRN2", target_bir_lowering=False, debug=False,
                   num_devices=N_CORES)
    q_d = nc.dram_tensor("q", [HEADS_PER_CORE, S, D], BF16, kind="ExternalInput").ap()
    k_d = nc.dram_tensor("k", [HEADS_PER_CORE, S, D], BF16, kind="ExternalInput").ap()
    v_d = nc.dram_tensor("v", [HEADS_PER_CORE, S, DV], BF16, kind="ExternalInput").ap()
    o_d = nc.dram_tensor("out", [HEADS_PER_CORE, S, D], F32, kind="ExternalOutput").ap()

    with tile.TileContext(nc) as tc:
        with tc.tile_pool(name="consts", bufs=1) as consts, \
             tc.tile_pool(name="inb", bufs=3) as inb, \
             tc.tile_pool(name="px", bufs=4) as px, \
             tc.tile_pool(name="sm", bufs=4) as sm, \
             tc.tile_pool(name="ps_a", bufs=1, space="PSUM") as ps_a, \
             tc.tile_pool(name="ps_b", bufs=1, space="PSUM") as ps_b, \
             tc.tile_pool(name="ps_o", bufs=1, space="PSUM") as ps_o:

            utm = consts.tile([P, P], BF16)  # utm[k,q] = 1 iff q >= k
            make_upper_triangular(nc, utm, val=1.0, diag=True)

            head_state = {}

            def emit_load(hh):
                qT = inb.tile([P, S], BF16, tag="qT", name="qT")
                kT = inb.tile([P, S], BF16, tag="kT", name="kT")
                vn = inb.tile([P, N_KT, DV], BF16, tag="vn", name="vn")
                # whole-tensor loads, ALL on the sync queue: transposing DMAs
                # issued from the scalar hwdge queue produce corrupted reads
                # (verified on HW — rel err 0.5), as do just-in-time split
                # pieces; with 2-head prefetch each load has ~30us of margin
                nc.sync.dma_start_transpose(out=kT, in_=k_d[hh])
                nc.sync.dma_start_transpose(out=qT, in_=q_d[hh])
                nc.sync.dma_start(
                    out=vn, in_=v_d[hh].rearrange("(t p) d -> p t d", p=P))
                head_state[hh] = dict(qT=qT, kT=kT, vn=vn)

            def make_pv(hh, c, offs, pexp, pso, final):
                st = head_state[hh]

                def emit():
                    for (s, j, oj) in offs:
                        t0 = max(0, j - 4 * c)
                        for t in range(t0, 4):
                            bank_first = ((t == 3 and j == 4 * c + 3) or
                                          (t == 2 and j == 4 * c + 2))
                            nc.tensor.matmul(
                                pso[:, PSO_OFF[t]:PSO_OFF[t] + 129],
                                pexp[:, s * CHUNK + t * P:s * CHUNK + (t + 1) * P],
                                st["vn"][:, j, 0:129],
                                start=bank_first, stop=(j == 0),
                                skip_group_check=True)
                    if final:
                        emit_tail(hh, c, pso,
                                  store_sync=(hh == HEADS_PER_CORE - 1))
                return emit

            def emit_tail(hh, c, pso, store_sync=False):
                # denominators live at psO cols 128,260,392,640
                den4 = sm.tile([P, 4], F32, tag="den4", name="den4")
                nc.vector.tensor_copy(
                    den4[:, 0:3],
                    pso[:, 128:524].rearrange("p (a b) -> p a b", b=132)[:, :, 0])
                nc.vector.tensor_copy(den4[:, 3:4], pso[:, 640:641])
                rc4 = sm.tile([P, 4], F32, tag="rc4", name="rc4")
                nc.vector.reciprocal_approx_fast(rc4, den4)
                outf = sm.tile([P, 4, P], F32, tag="outf", name="outf")
                for t in range(4):
                    nc.vector.tensor_scalar_mul(
                        outf[:, t, :], pso[:, PSO_OFF[t]:PSO_OFF[t] + P],
                        rc4[:, t:t + 1])
                # stores go via gpsimd swdge (keeps them off the sync queue so
                # they never alias loads' DMA semaphores) except for the last
                # head, where sync is idle and drains faster
                eng = nc.sync if store_sync else nc.gpsimd
                eng.dma_start(
                    out=o_d[hh, CHUNK * c:CHUNK * (c + 1), :].rearrange(
                        "(t p) d -> p t d", p=P),
                    in_=outf)

            emit_load(0)
            emit_load(1)
            # PE warm-up: the PE would otherwise idle ~6us waiting for the
            # first transposed loads and start cold (HAM K=4/8, 1.2GHz).
            # ~36 dummy matmuls on the utm constant (~3.9us busy) trip the
            # activity monitor to full clock before the first real QK.
            # Output goes to the psA slot and is overwritten by the first
            # real group's start=True.
            warm = ps_a.tile([P, 2048], F32, tag="a", name="warm")
            for _ in range(36):
                nc.tensor.matmul(warm[:, 0:P], utm, utm, start=True, stop=True)
            pending = deque()  # PV closures, deferred by 2 groups
            use_a = True       # global psA/psB alternation (never adjacent)
            for hh in range(HEADS_PER_CORE):
                st = head_state[hh]
                if hh + 2 < HEADS_PER_CORE:
                    emit_load(hh + 2)

                # end every head on a small chunk: the PV backlog of a big
                # chunk colliding with the next head's small first act causes
                # ACT bubbles at head boundaries (and a long drain at the end)
                chunk_order = [0, 1, 2, 3] if hh == 0 else [1, 2, 3, 0]
                for c in chunk_order:
                    jmax = 4 * c + 3
                    pso = ps_o.tile([P, 1024], F32, tag="pso", name="pso")
                    # descending-j groups (diag tiles first, descending oj so
                    # the merged exp can skip the leading trimmed columns);
                    # psA(4-tile)/psB(2-tile) strictly alternate globally
                    js = list(range(jmax, -1, -1))
                    groups = []
                    ga = use_a
                    while js:
                        n = min(4 if ga else 2, len(js))
                        groups.append(js[:n])
                        js = js[n:]
                        ga = not ga

                    for gi, js_g in enumerate(groups):
                        pool = ps_a if use_a else ps_b
                        width = 2048 if use_a else 1024
                        psum = pool.tile([P, width], F32,
                                         tag="a" if use_a else "b",
                                         name="ps")
                        use_a = not use_a
                        pexp = px.tile([P, 2048], BF16, tag="pexp", name="pexp")
                        offs = []
                        for s, j in enumerate(js_g):
                            oj = max(0, P * j - CHUNK * c)
                            offs.append((s, j, oj))
                            nc.tensor.matmul(
                                psum[:, s * CHUNK + oj:(s + 1) * CHUNK],
                                st["kT"][:, j * P:(j + 1) * P],
                                st["qT"][:, CHUNK * c + oj:CHUNK * (c + 1)],
                                start=True, stop=True)
                        a0 = offs[0][2]
                        gw = len(js_g) * CHUNK
                        nc.scalar.activation(
                            pexp[:, a0:gw], psum[:, a0:gw],
                            EXP, scale=1.0 / TEMPERATURE)
                        for (s, j, oj) in offs:
                            ojb = P * j - CHUNK * c
                            if ojb >= 0:  # diagonal 128-block: mask q < k
                                sl = slice(s * CHUNK + ojb, s * CHUNK + ojb + P)
                                nc.vector.tensor_mul(pexp[:, sl], pexp[:, sl],
                                                     utm)
                        pending.append(make_pv(hh, c, offs, pexp, pso,
                                               final=(gi == len(groups) - 1)))
                        while len(pending) > 2:
                            pending.popleft()()
            # flush the last deferred groups
            while pending:
                pending.popleft()()

    nc.compile()
    return nc


_NC_CACHE = None


def _get_nc():
    global _NC_CACHE
    if _NC_CACHE is None:
        _NC_CACHE = build_attention_nc()
    return _NC_CACHE


def kernel(q, k, v, mask=None, _trace=False):
    """Full-input entry point: q,k,v [2,16,2048,128] f32, mask [2,1,2048,2048]
    int32 (causal; the kernel hardcodes causality and does not read it).
    Returns [2,16,2048,128] f32."""
    nc = _get_nc()
    bf = ml_dtypes.bfloat16
    qf = np.ascontiguousarray(
        np.asarray(q, dtype=np.float32).reshape(B * H, S, D)).astype(bf)
    kf = np.ascontiguousarray(
        np.asarray(k, dtype=np.float32).reshape(B * H, S, D)).astype(bf)
    vf = np.asarray(v, dtype=np.float32).reshape(B * H, S, D)
    v1 = np.empty((B * H, S, DV), dtype=bf)
    v1[:, :, 0:D] = vf.astype(bf)
    v1[:, :, D] = 1.0
    v1[:, :, D + 1:] = 0.0
    in_maps = []
    for i in range(N_CORES):
        sl = slice(i * HEADS_PER_CORE, (i + 1) * HEADS_PER_CORE)
        in_maps.append({"q": np.ascontiguousarray(qf[sl]),
                        "k": np.ascontiguousarray(kf[sl]),
                        "v": np.ascontiguousarray(v1[sl])})
    res = run_bass_kernel_spmd(nc, in_maps, list(range(N_CORES)), trace=_trace)
    out = np.concatenate([res.results[i]["out"] for i in range(N_CORES)], axis=0)
    out = out.reshape(B, H, S, D).astype(np.float32)
    if _trace:
        return out, res
    return out


# revision 36
# speedup vs baseline: 1.0053x; 1.0053x over previous
"""Causal scaled-dot-product attention for Trainium2 (Bass/Tile), 8-core SPMD.

Problem: B=2, H=16, S=2048, D=128 fp32, causal mask, softmax(QK^T/sqrt(D)) @ V.
Sharding: batch*heads (32) split across 8 cores, 4 heads per core; attention is
independent per (b,h) so there is no communication.

Design (bf16 on the PE; ~1.6x over the f32r predecessor, ~109-113us):
  - Host casts Q,K,V to bf16 and appends a ones-column to V (V1 = [V | 1]).
  - Q^T,K^T loaded straight into SBUF via 2-byte DMA xbar transpose
    (dma_start_transpose) -> zero PE transposes.  Whole-tensor loads with
    2-head prefetch only: just-in-time split pieces raced on HW.
  - Per 512-wide query chunk, key tiles are processed in descending-j groups
    of 4 (psA, 4 PSUM banks) strictly alternating with 2 (psB, 2 banks):
      S^T[j] = K_j @ Q_c^T          (bf16 matmul, 1 col/cycle; fp32r is 2)
      one merged exp per group      (ACT, PSUM->SBUF bf16; trimmed cols of
                                     later slots exp junk that is never read)
      diagonal 128-blocks masked in place on DVE with a bf16 upper-tri const
  - PV uses pexp as the *stationary* operand and V1 as the moving operand:
      OUT[qtile, 0:129] += pexp_j,t^T @ [V_j | 1]
    so the output lands directly in [q, d] layout (no output transpose) and
    column 128 accumulates the softmax denominator for free.
  - Tail per chunk: reciprocal of den, per-partition scale, DMA out (gpsimd
    swdge queue, so stores never alias the sync queue's load semaphores).
Softmax max-subtraction is skipped: logits are bounded (~±6) so exp is safe,
and softmax is shift-invariant.

Steady state is ACT(exp)-bound (~77us busy) with the PE close behind (~72us,
PV is LDWEIGHTS-rate-limited); PV is deferred by exactly two groups (three
broke: intermittent races), heads end on a small chunk to avoid PV-backlog
bubbles at head boundaries, and the whole kernel drains on chunk c0.
"""
from collections import deque

import numpy as np
import ml_dtypes

import concourse.bacc as bacc
import concourse.tile as tile
import concourse.mybir as mybir
from concourse.bass_utils import run_bass_kernel_spmd
from concourse.masks import make_upper_triangular

F32 = mybir.dt.float32
BF16 = mybir.dt.bfloat16
EXP = mybir.ActivationFunctionType.Exp

B, H, S, D = 2, 16, 2048, 128
TEMPERATURE = 11.313708498984761  # sqrt(128)
N_CORES = 8
HEADS_PER_CORE = (B * H) // N_CORES  # 4
P = 128                    # partitions / tile edge
CHUNK = 512                # query chunk
N_KT = S // P              # 16 key tiles per head
N_CH = S // CHUNK          # 4 query chunks per head
DV = 132                   # V free size: 128 d + 1 ones + 3 pad
# psO slot layout: per-qtile [q,129] accumulation regions, each within a
# single 2KB PSUM bank (bank0: t0..t2, bank1: t3).  start_tensor_calc marks
# the WHOLE bank pending-zero, so start=True is only emitted on the first
# write to each bank per chunk (t3's and t2's diag matmuls); first writes to
# the other regions rely on the bank-wide pending-zero to land as overwrites.
PSO_OFF = (0, 132, 264, 512)


def build_attention_nc():
    nc = bacc.Bacc("TRN2", target_bir_lowering=False, debug=False,
                   num_devices=N_CORES)
    q_d = nc.dram_tensor("q", [HEADS_PER_CORE, S, D], BF16, kind="ExternalInput").ap()
    k_d = nc.dram_tensor("k", [HEADS_PER_CORE, S, D], BF16, kind="ExternalInput").ap()
    v_d = nc.dram_tensor("v", [HEADS_PER_CORE, S, DV], BF16, kind="ExternalInput").ap()
    o_d = nc.dram_tensor("out", [HEADS_PER_CORE, S, D], F32, kind="ExternalOutput").ap()

    with tile.TileContext(nc) as tc:
        with tc.tile_pool(name="consts", bufs=1) as consts, \
             tc.tile_pool(name="inb", bufs=3) as inb, \
             tc.tile_pool(name="px", bufs=4) as px, \
             tc.tile_pool(name="sm", bufs=4) as sm, \
             tc.tile_pool(name="ps_a", bufs=1, space="PSUM") as ps_a, \
             tc.tile_pool(name="ps_b", bufs=1, space="PSUM") as ps_b, \
             tc.tile_pool(name="ps_o", bufs=1, space="PSUM") as ps_o:

            utm = consts.tile([P, P], BF16)  # utm[k,q] = 1 iff q >= k
            make_upper_triangular(nc, utm, val=1.0, diag=True)

            head_state = {}

            def emit_load(hh):
                qT = inb.tile([P, S], BF16, tag="qT", name="qT")
                kT = inb.tile([P, S], BF16, tag="kT", name="kT")
                vn = inb.tile([P, N_KT, DV], BF16, tag="vn", name="vn")
                # whole-tensor loads, ALL on the sync queue: transposing DMAs
                # issued from the scalar hwdge queue produce corrupted reads
                # (verified on HW — rel err 0.5), as do just-in-time split
                # pieces; with 2-head prefetch each load has ~30us of margin
                nc.sync.dma_start_transpose(out=kT, in_=k_d[hh])
                nc.sync.dma_start_transpose(out=qT, in_=q_d[hh])
                nc.sync.dma_start(
                    out=vn, in_=v_d[hh].rearrange("(t p) d -> p t d", p=P))
                head_state[hh] = dict(qT=qT, kT=kT, vn=vn)

            def make_pv(hh, c, offs, pexp, pso, final):
                st = head_state[hh]

                def emit():
                    for (s, j, oj) in offs:
                        t0 = max(0, j - 4 * c)
                        for t in range(t0, 4):
                            bank_first = ((t == 3 and j == 4 * c + 3) or
                                          (t == 2 and j == 4 * c + 2))
                            nc.tensor.matmul(
                                pso[:, PSO_OFF[t]:PSO_OFF[t] + 129],
                                pexp[:, s * CHUNK + t * P:s * CHUNK + (t + 1) * P],
                                st["vn"][:, j, 0:129],
                                start=bank_first, stop=(j == 0),
                                skip_group_check=True)
                    if final:
                        emit_tail(hh, c, pso,
                                  store_sync=(hh == HEADS_PER_CORE - 1))
                return emit

            def emit_tail(hh, c, pso, store_sync=False):
                # denominators live at psO cols 128,260,392,640
                den4 = sm.tile([P, 4], F32, tag="den4", name="den4")
                nc.vector.tensor_copy(
                    den4[:, 0:3],
                    pso[:, 128:524].rearrange("p (a b) -> p a b", b=132)[:, :, 0])
                nc.vector.tensor_copy(den4[:, 3:4], pso[:, 640:641])
                rc4 = sm.tile([P, 4], F32, tag="rc4", name="rc4")
                nc.vector.reciprocal_approx_fast(rc4, den4)
                outf = sm.tile([P, 4, P], F32, tag="outf", name="outf")
                for t in range(4):
                    nc.vector.tensor_scalar_mul(
                        outf[:, t, :], pso[:, PSO_OFF[t]:PSO_OFF[t] + P],
                        rc4[:, t:t + 1])
                # stores go via gpsimd swdge (keeps them off the sync queue so
                # they never alias loads' DMA semaphores) except for the last
                # head, where sync is idle and drains faster
                eng = nc.sync if store_sync else nc.gpsimd
                eng.dma_start(
                    out=o_d[hh, CHUNK * c:CHUNK * (c + 1), :].rearrange(
                        "(t p) d -> p t d", p=P),
                    in_=outf)

            emit_load(0)
            emit_load(1)
            # PE warm-up: the PE would otherwise idle ~6us waiting for the
            # first transposed loads and start cold (HAM K=4/8, 1.2GHz).
            # ~36 dummy matmuls on the utm constant (~3.9us busy) trip the
            # activity monitor to full clock before the first real QK.
            # Output goes to the psA slot and is overwritten by the first
            # real group's start=True.
            warm = ps_a.tile([P, 2048], F32, tag="a", name="warm")
            for _ in range(64):
                nc.tensor.matmul(warm[:, 0:P], utm, utm, start=True, stop=True)
            pending = deque()  # PV closures, deferred by 2 groups
            use_a = True       # global psA/psB alternation (never adjacent)
            for hh in range(HEADS_PER_CORE):
                st = head_state[hh]
                if hh + 2 < HEADS_PER_CORE:
                    emit_load(hh + 2)

                # end every head on a small chunk: the PV backlog of a big
                # chunk colliding with the next head's small first act causes
                # ACT bubbles at head boundaries (and a long drain at the end)
                chunk_order = [0, 1, 2, 3] if hh == 0 else [1, 2, 3, 0]
                for c in chunk_order:
                    jmax = 4 * c + 3
                    pso = ps_o.tile([P, 1024], F32, tag="pso", name="pso")
                    # descending-j groups (diag tiles first, descending oj so
                    # the merged exp can skip the leading trimmed columns);
                    # psA(4-tile)/psB(2-tile) strictly alternate globally
                    js = list(range(jmax, -1, -1))
                    groups = []
                    ga = use_a
                    while js:
                        n = min(4 if ga else 2, len(js))
                        groups.append(js[:n])
                        js = js[n:]
                        ga = not ga

                    for gi, js_g in enumerate(groups):
                        pool = ps_a if use_a else ps_b
                        width = 2048 if use_a else 1024
                        psum = pool.tile([P, width], F32,
                                         tag="a" if use_a else "b",
                                         name="ps")
                        use_a = not use_a
                        pexp = px.tile([P, 2048], BF16, tag="pexp", name="pexp")
                        offs = []
                        for s, j in enumerate(js_g):
                            oj = max(0, P * j - CHUNK * c)
                            offs.append((s, j, oj))
                            nc.tensor.matmul(
                                psum[:, s * CHUNK + oj:(s + 1) * CHUNK],
                                st["kT"][:, j * P:(j + 1) * P],
                                st["qT"][:, CHUNK * c + oj:CHUNK * (c + 1)],
                                start=True, stop=True)
                        a0 = offs[0][2]
                        gw = len(js_g) * CHUNK
                        nc.scalar.activation(
                            pexp[:, a0:gw], psum[:, a0:gw],
                            EXP, scale=1.0 / TEMPERATURE)
                        for (s, j, oj) in offs:
                            ojb = P * j - CHUNK * c
                            if ojb >= 0:  # diagonal 128-block: mask q < k
                                sl = slice(s * CHUNK + ojb, s * CHUNK + ojb + P)
                                nc.vector.tensor_mul(pexp[:, sl], pexp[:, sl],
                                                     utm)
                        pending.append(make_pv(hh, c, offs, pexp, pso,
                                               final=(gi == len(groups) - 1)))
                        while len(pending) > 2:
                            pending.popleft()()
            # flush the last deferred groups
            while pending:
                pending.popleft()()

    nc.compile()
    return nc


_NC_CACHE = None


def _get_nc():
    global _NC_CACHE
    if _NC_CACHE is None:
        _NC_CACHE = build_attention_nc()
    return _NC_CACHE


def kernel(q, k, v, mask=None, _trace=False):
    """Full-input entry point: q,k,v [2,16,2048,128] f32, mask [2,1,2048,2048]
    int32 (causal; the kernel hardcodes causality and does not read it).
    Returns [2,16,2048,128] f32."""
    nc = _get_nc()
    bf = ml_dtypes.bfloat16
    qf = np.ascontiguousarray(
        np.asarray(q, dtype=np.float32).reshape(B * H, S, D)).astype(bf)
    kf = np.ascontiguousarray(
        np.asarray(k, dtype=np.float32).reshape(B * H, S, D)).astype(bf)
    vf = np.asarray(v, dtype=np.float32).reshape(B * H, S, D)
    v1 = np.empty((B * H, S, DV), dtype=bf)
    v1[:, :, 0:D] = vf.astype(bf)
    v1[:, :, D] = 1.0
    v1[:, :, D + 1:] = 0.0
    in_maps = []
    for i in range(N_CORES):
        sl = slice(i * HEADS_PER_CORE, (i + 1) * HEADS_PER_CORE)
        in_maps.append({"q": np.ascontiguousarray(qf[sl]),
                        "k": np.ascontiguousarray(kf[sl]),
                        "v": np.ascontiguousarray(v1[sl])})
    res = run_bass_kernel_spmd(nc, in_maps, list(range(N_CORES)), trace=_trace)
    out = np.concatenate([res.results[i]["out"] for i in range(N_CORES)], axis=0)
    out = out.reshape(B, H, S, D).astype(np.float32)
    if _trace:
        return out, res
    return out


# revision 37
# speedup vs baseline: 1.0361x; 1.0306x over previous
"""Causal scaled-dot-product attention for Trainium2 (Bass/Tile), 8-core SPMD.

Problem: B=2, H=16, S=2048, D=128 fp32, causal mask, softmax(QK^T/sqrt(D)) @ V.
Sharding: batch*heads (32) split across 8 cores, 4 heads per core; attention is
independent per (b,h) so there is no communication.

Design (bf16 on the PE; ~1.6x over the f32r predecessor, ~109-113us):
  - Host casts Q,K,V to bf16 and appends a ones-column to V (V1 = [V | 1]).
  - Q^T,K^T loaded straight into SBUF via 2-byte DMA xbar transpose
    (dma_start_transpose) -> zero PE transposes.  Whole-tensor loads with
    2-head prefetch only: just-in-time split pieces raced on HW.
  - Per 512-wide query chunk, key tiles are processed in descending-j groups
    of 4 (psA, 4 PSUM banks) strictly alternating with 2 (psB, 2 banks):
      S^T[j] = K_j @ Q_c^T          (bf16 matmul, 1 col/cycle; fp32r is 2)
      one merged exp per group      (ACT, PSUM->SBUF bf16; trimmed cols of
                                     later slots exp junk that is never read)
      diagonal 128-blocks masked in place on DVE with a bf16 upper-tri const
  - PV uses pexp as the *stationary* operand and V1 as the moving operand:
      OUT[qtile, 0:129] += pexp_j,t^T @ [V_j | 1]
    so the output lands directly in [q, d] layout (no output transpose) and
    column 128 accumulates the softmax denominator for free.
  - Tail per chunk: reciprocal of den, per-partition scale, DMA out (gpsimd
    swdge queue, so stores never alias the sync queue's load semaphores).
Softmax max-subtraction is skipped: logits are bounded (~±6) so exp is safe,
and softmax is shift-invariant.

Steady state is ACT(exp)-bound (~77us busy) with the PE close behind (~72us,
PV is LDWEIGHTS-rate-limited); PV is deferred by exactly two groups (three
broke: intermittent races), heads end on a small chunk to avoid PV-backlog
bubbles at head boundaries, and the whole kernel drains on chunk c0.
"""
from collections import deque

import numpy as np
import ml_dtypes

import concourse.bacc as bacc
import concourse.tile as tile
import concourse.mybir as mybir
from concourse.bass_utils import run_bass_kernel_spmd
from concourse.masks import make_upper_triangular

F32 = mybir.dt.float32
BF16 = mybir.dt.bfloat16
EXP = mybir.ActivationFunctionType.Exp

B, H, S, D = 2, 16, 2048, 128
TEMPERATURE = 11.313708498984761  # sqrt(128)
N_CORES = 8
HEADS_PER_CORE = (B * H) // N_CORES  # 4
P = 128                    # partitions / tile edge
CHUNK = 512                # query chunk
N_KT = S // P              # 16 key tiles per head
N_CH = S // CHUNK          # 4 query chunks per head
DV = 132                   # V free size: 128 d + 1 ones + 3 pad
# psO slot layout: per-qtile [q,129] accumulation regions, each within a
# single 2KB PSUM bank (bank0: t0..t2, bank1: t3).  start_tensor_calc marks
# the WHOLE bank pending-zero, so start=True is only emitted on the first
# write to each bank per chunk (t3's and t2's diag matmuls); first writes to
# the other regions rely on the bank-wide pending-zero to land as overwrites.
PSO_OFF = (0, 132, 264, 512)


def build_attention_nc():
    nc = bacc.Bacc("TRN2", target_bir_lowering=False, debug=False,
                   num_devices=N_CORES)
    qk_d = nc.dram_tensor("qk", [HEADS_PER_CORE, 2 * S, D], BF16,
                          kind="ExternalInput").ap()  # [K rows; Q rows]
    v_d = nc.dram_tensor("v", [HEADS_PER_CORE, S, DV], BF16, kind="ExternalInput").ap()
    o_d = nc.dram_tensor("out", [HEADS_PER_CORE, S, D], F32, kind="ExternalOutput").ap()

    with tile.TileContext(nc) as tc:
        with tc.tile_pool(name="consts", bufs=1) as consts, \
             tc.tile_pool(name="inb", bufs=3) as inb, \
             tc.tile_pool(name="px", bufs=4) as px, \
             tc.tile_pool(name="sm", bufs=4) as sm, \
             tc.tile_pool(name="ps_a", bufs=1, space="PSUM") as ps_a, \
             tc.tile_pool(name="ps_b", bufs=1, space="PSUM") as ps_b, \
             tc.tile_pool(name="ps_o", bufs=1, space="PSUM") as ps_o:

            utm = consts.tile([P, P], BF16)  # utm[k,q] = 1 iff q >= k
            make_upper_triangular(nc, utm, val=1.0, diag=True)

            head_state = {}

            def emit_load(hh):
                qkT = inb.tile([P, 2 * S], BF16, tag="qkT", name="qkT")
                vn = inb.tile([P, N_KT, DV], BF16, tag="vn", name="vn")
                # ONE whole-tensor transpose per head (K and Q concatenated on
                # the host), issued on the sync queue only: transposing DMAs
                # from the scalar hwdge queue corrupt reads (verified on HW),
                # as do just-in-time split pieces; 2-head prefetch gives
                # ~30us of completion margin
                nc.sync.dma_start_transpose(out=qkT, in_=qk_d[hh])
                nc.sync.dma_start(
                    out=vn, in_=v_d[hh].rearrange("(t p) d -> p t d", p=P))
                head_state[hh] = dict(qT=qkT[:, S:2 * S], kT=qkT[:, 0:S],
                                      vn=vn)

            def make_pv(hh, c, offs, pexp, pso, final):
                st = head_state[hh]

                def emit():
                    for (s, j, oj) in offs:
                        t0 = max(0, j - 4 * c)
                        for t in range(t0, 4):
                            bank_first = ((t == 3 and j == 4 * c + 3) or
                                          (t == 2 and j == 4 * c + 2))
                            nc.tensor.matmul(
                                pso[:, PSO_OFF[t]:PSO_OFF[t] + 129],
                                pexp[:, s * CHUNK + t * P:s * CHUNK + (t + 1) * P],
                                st["vn"][:, j, 0:129],
                                start=bank_first, stop=(j == 0),
                                skip_group_check=True)
                    if final:
                        emit_tail(hh, c, pso,
                                  store_sync=(hh == HEADS_PER_CORE - 1))
                return emit

            def emit_tail(hh, c, pso, store_sync=False):
                # denominators live at psO cols 128,260,392,640
                den4 = sm.tile([P, 4], F32, tag="den4", name="den4")
                nc.vector.tensor_copy(
                    den4[:, 0:3],
                    pso[:, 128:524].rearrange("p (a b) -> p a b", b=132)[:, :, 0])
                nc.vector.tensor_copy(den4[:, 3:4], pso[:, 640:641])
                rc4 = sm.tile([P, 4], F32, tag="rc4", name="rc4")
                nc.vector.reciprocal_approx_fast(rc4, den4)
                outf = sm.tile([P, 4, P], F32, tag="outf", name="outf")
                for t in range(4):
                    nc.vector.tensor_scalar_mul(
                        outf[:, t, :], pso[:, PSO_OFF[t]:PSO_OFF[t] + P],
                        rc4[:, t:t + 1])
                # stores go via gpsimd swdge (keeps them off the sync queue so
                # they never alias loads' DMA semaphores) except for the last
                # head, where sync is idle and drains faster
                eng = nc.sync if store_sync else nc.gpsimd
                eng.dma_start(
                    out=o_d[hh, CHUNK * c:CHUNK * (c + 1), :].rearrange(
                        "(t p) d -> p t d", p=P),
                    in_=outf)

            emit_load(0)
            emit_load(1)
            # PE warm-up: the PE would otherwise idle ~6us waiting for the
            # first transposed loads and start cold (HAM K=4/8, 1.2GHz).
            # ~36 dummy matmuls on the utm constant (~3.9us busy) trip the
            # activity monitor to full clock before the first real QK.
            # Output goes to the psA slot and is overwritten by the first
            # real group's start=True.
            warm = ps_a.tile([P, 2048], F32, tag="a", name="warm")
            for _ in range(64):
                nc.tensor.matmul(warm[:, 0:P], utm, utm, start=True, stop=True)
            pending = deque()  # PV closures, deferred by 2 groups
            use_a = True       # global psA/psB alternation (never adjacent)
            for hh in range(HEADS_PER_CORE):
                st = head_state[hh]
                if hh + 2 < HEADS_PER_CORE:
                    emit_load(hh + 2)

                # end every head on a small chunk: the PV backlog of a big
                # chunk colliding with the next head's small first act causes
                # ACT bubbles at head boundaries (and a long drain at the end)
                chunk_order = ([0, 1, 2, 3] if hh == 0 else
                               [2, 3, 1, 0] if hh == HEADS_PER_CORE - 1 else
                               [1, 2, 3, 0])
                for c in chunk_order:
                    jmax = 4 * c + 3
                    pso = ps_o.tile([P, 1024], F32, tag="pso", name="pso")
                    # descending-j groups (diag tiles first, descending oj so
                    # the merged exp can skip the leading trimmed columns);
                    # psA(4-tile)/psB(2-tile) strictly alternate globally
                    js = list(range(jmax, -1, -1))
                    groups = []
                    ga = use_a
                    while js:
                        n = min(4 if ga else 2, len(js))
                        groups.append(js[:n])
                        js = js[n:]
                        ga = not ga

                    for gi, js_g in enumerate(groups):
                        pool = ps_a if use_a else ps_b
                        width = 2048 if use_a else 1024
                        psum = pool.tile([P, width], F32,
                                         tag="a" if use_a else "b",
                                         name="ps")
                        use_a = not use_a
                        pexp = px.tile([P, 2048], BF16, tag="pexp", name="pexp")
                        offs = []
                        for s, j in enumerate(js_g):
                            oj = max(0, P * j - CHUNK * c)
                            offs.append((s, j, oj))
                            nc.tensor.matmul(
                                psum[:, s * CHUNK + oj:(s + 1) * CHUNK],
                                st["kT"][:, j * P:(j + 1) * P],
                                st["qT"][:, CHUNK * c + oj:CHUNK * (c + 1)],
                                start=True, stop=True)
                        a0 = offs[0][2]
                        gw = len(js_g) * CHUNK
                        nc.scalar.activation(
                            pexp[:, a0:gw], psum[:, a0:gw],
                            EXP, scale=1.0 / TEMPERATURE)
                        for (s, j, oj) in offs:
                            ojb = P * j - CHUNK * c
                            if ojb >= 0:  # diagonal 128-block: mask q < k
                                sl = slice(s * CHUNK + ojb, s * CHUNK + ojb + P)
                                nc.vector.tensor_mul(pexp[:, sl], pexp[:, sl],
                                                     utm)
                        pending.append(make_pv(hh, c, offs, pexp, pso,
                                               final=(gi == len(groups) - 1)))
                        while len(pending) > 2:
                            pending.popleft()()
            # flush the last deferred groups
            while pending:
                pending.popleft()()

    nc.compile()
    return nc


_NC_CACHE = None


def _get_nc():
    global _NC_CACHE
    if _NC_CACHE is None:
        _NC_CACHE = build_attention_nc()
    return _NC_CACHE


def kernel(q, k, v, mask=None, _trace=False):
    """Full-input entry point: q,k,v [2,16,2048,128] f32, mask [2,1,2048,2048]
    int32 (causal; the kernel hardcodes causality and does not read it).
    Returns [2,16,2048,128] f32."""
    nc = _get_nc()
    bf = ml_dtypes.bfloat16
    qf = np.asarray(q, dtype=np.float32).reshape(B * H, S, D).astype(bf)
    kf = np.asarray(k, dtype=np.float32).reshape(B * H, S, D).astype(bf)
    qkf = np.concatenate([kf, qf], axis=1)  # [BH, 2S, D], K rows then Q
    vf = np.asarray(v, dtype=np.float32).reshape(B * H, S, D)
    v1 = np.empty((B * H, S, DV), dtype=bf)
    v1[:, :, 0:D] = vf.astype(bf)
    v1[:, :, D] = 1.0
    v1[:, :, D + 1:] = 0.0
    in_maps = []
    for i in range(N_CORES):
        sl = slice(i * HEADS_PER_CORE, (i + 1) * HEADS_PER_CORE)
        in_maps.append({"qk": np.ascontiguousarray(qkf[sl]),
                        "v": np.ascontiguousarray(v1[sl])})
    res = run_bass_kernel_spmd(nc, in_maps, list(range(N_CORES)), trace=_trace)
    out = np.concatenate([res.results[i]["out"] for i in range(N_CORES)], axis=0)
    out = out.reshape(B, H, S, D).astype(np.float32)
    if _trace:
        return out, res
    return out
